# revision 20
# baseline (speedup 1.0000x reference)
"""Unfold/im2col kernel for Trainium2 (Bass/Tile), 8-core data parallel.

Problem: x [4, 64, 224, 224] f32 -> out [4, 576, 49729] f32 where
out[b, (c*3+kh)*3+kw, oh*223+ow] = pad(x,1)[b, c, oh+kh, ow+kw]
(3x3 kernel, pad 1, stride 1, dilation 1, oh=ow=223).

Sharding: 8 cores = (batch 4) x (channel half 2). Each core handles
32 channels -> [288, 49729] independently; outputs concatenate on the
channel axis (channel-major row layout makes halves contiguous).

Design notes (v4). The baseline (330 us) was SDMA-descriptor-bound:
each store descriptor was one 892 B output row. Three levers fix it:

 * bf16 stores: the 2e-2 rel-err budget dwarfs bf16's 2^-9 rounding
   (measured 3e-3); host pre-casts the input and upcasts the gather.
   Halves HBM write traffic to 28.6 MB/core.
 * On-chip repack so descriptors are ~25 KB: the Vector/Scalar engines
   pack 3 kw-crops with row stride 223 (= output row length); any
   (kh,kw) plane chunk is then contiguous in a partition's free dim
   AND in DRAM.
 * Few DMAs with MANY descriptors: the SWDGE model queue executes
   DMAs serially; within a DMA its descriptors round-robin over the
   16 SDMA engines, and each engine pipelines its own descriptor
   chain.  1 descriptor/engine/DMA measures ~3.4 us per 25 KB packet
   (latency-bound); 6/engine hides it.

Layout per core: partition p = rb*32 + c holds row-block rb of
channel c. Row-blocks OVERLAP: block rb = padded rows [57rb, 57rb+58]
(59 rows; host pads H to 230 = 1 top + 5 bottom zero rows). The
overlap makes every (kh, rb) store chunk a uniform 57 output rows
(52 for rb=3) starting at local row kh, so kh becomes a middle AP
dim: one store DMA per (kw, rb) = 32 channels x 3 kh = 96
descriptors of ~25 KB. 4 load DMAs (one per rb, 32 x 26.7 KB
contiguous descriptors), 3 crop copies (kw=0,2 on DVE, kw=1 on the
otherwise-idle Scalar engine), 12 store DMAs.

HBM traffic/core: 3.4 MB read + 28.6 MB write.
"""

from contextlib import ExitStack

import ml_dtypes
import numpy as np

import concourse.bass as bass
import concourse.tile as tile
from concourse import mybir
from concourse.ap import AP
from concourse.bass_utils import run_bass_kernel_spmd

B, C, IH, IW = 4, 64, 224, 224
N_CORES = 8
CPC = C // 2          # channels per core: 32
PW = IW + 2           # padded width: 226
PH2 = IH + 6          # padded height incl. 5 bottom zero rows: 230
OH = IH - 1           # output spatial: 223
OSZ = OH * OH         # 49729
NROW = CPC * 9        # 288 output rows per core
RB = 4                # row-blocks per channel
RBH = 57              # block start stride (rows)
BLK = RBH + 2         # rows per block incl. 2-row overlap: 59
FRAW = BLK * PW       # 13334 raw elems per partition
FCROP = BLK * OH      # 13157 crop elems per partition
NPART = RB * CPC      # 128 partitions used
NP_DT = ml_dtypes.bfloat16
BIR_DT = mybir.dt.bfloat16

_NC_CACHE = {}


def build_nc() -> bass.Bass:
    nc = bass.Bass()
    x = nc.declare_dram_parameter("xp", [CPC, PH2, PW], BIR_DT, isOutput=False)
    out = nc.declare_dram_parameter("out", [NROW, OSZ], BIR_DT, isOutput=True)
    xb = x[:, :, :]
    ob = out[:, :]

    with tile.TileContext(nc) as tc:
        with ExitStack() as ctx:
            pool = ctx.enter_context(tc.tile_pool(name="img", bufs=1))
            raw = pool.tile([NPART, FRAW], BIR_DT, name="raw", tag="raw")[:, :]
            crops = [
                pool.tile([NPART, FCROP], BIR_DT, name=f"c{kw}", tag=f"c{kw}")[:, :]
                for kw in range(3)
            ]

            # Loads: one DMA per (row-half, rb) with CHANNELS as the
            # outer AP dim — SWDGE spreads descriptors over SDMA
            # engines by outer-dim index, so the outer count must be
            # >= 16 (an rb-outer single DMA measured 4-engines-only,
            # 60 us). Partition p = rb*32+c gets rows [57rb, 57rb+58]
            # of channel c (overlapping DRAM reads are fine). The
            # row-half split lets crop0's first half run on DVE while
            # the second half of the image still loads.
            HA = 30  # rows in first half
            for r0, nr in ((0, HA), (HA, BLK - HA)):
                for rb in range(RB):
                    nc.gpsimd.dma_start(
                        out=AP(
                            raw.tensor,
                            raw.offset + (rb * CPC) * FRAW + r0 * PW,
                            [[FRAW, CPC], [1, nr * PW]],
                        ),
                        in_=AP(
                            xb.tensor,
                            xb.offset + (rb * RBH + r0) * PW,
                            [[PH2 * PW, CPC], [1, nr * PW]],
                        ),
                    )

            # Shift-pack the 3 kw-crops (row stride 226 -> 223).
            # crop0 gates the first stores: split by rows over
            # DVE + Scalar. Stores for each kw are EMITTED right after
            # that kw's crop so any conservative queue-drain sync only
            # covers already-needed ops. kw=1 runs on Scalar, kw=2 on
            # DVE, both hidden under the kw=0 store burst.
            def cap(t, free, kw_off, pitch, r0, nr):
                return AP(
                    t.tensor,
                    t.offset + kw_off + r0 * (OH if pitch == OH else PW),
                    [[free, NPART], [pitch, nr], [1, OH]],
                )

            def emit_stores(kw):
                ck = crops[kw]
                for kh in range(3):
                    for rb in range(RB):
                        nrows = RBH if rb < RB - 1 else OH - RBH * (RB - 1)
                        h1 = (nrows + 1) // 2
                        for q0, qn in ((0, h1), (h1, nrows - h1)):
                            src = AP(
                                ck.tensor,
                                ck.offset + (rb * CPC) * FCROP + (kh + q0) * OH,
                                [[FCROP, CPC], [1, qn * OH]],
                            )
                            dst = AP(
                                ob.tensor,
                                ob.offset
                                + (kh * 3 + kw) * OSZ
                                + (rb * RBH + q0) * OH,
                                [[9 * OSZ, CPC], [1, qn * OH]],
                            )
                            nc.gpsimd.dma_start(out=dst, in_=src)

            # All crops single-writer on DVE: a tile written by two
            # ENGINES makes readers fall back to queue-drain syncs
            # (measured: first store waited for crop2's drain, +6 us).
            # Two DVE ops on one tile keep fine-grained op-count waits.
            # crop0's first half starts right after the first-half
            # loads, overlapping the second-half loads.
            nc.vector.tensor_copy(
                out=cap(crops[0], FCROP, 0, OH, 0, HA),
                in_=cap(raw, FRAW, 0, PW, 0, HA),
            )
            nc.vector.tensor_copy(
                out=cap(crops[0], FCROP, 0, OH, HA, BLK - HA),
                in_=cap(raw, FRAW, 0, PW, HA, BLK - HA),
            )
            emit_stores(0)
            for kw in (1, 2):
                nc.vector.tensor_copy(
                    out=cap(crops[kw], FCROP, 0, OH, 0, BLK),
                    in_=cap(raw, FRAW, kw, PW, 0, BLK),
                )
                emit_stores(kw)
    return nc


def _split_multi_waits(nc: bass.Bass) -> None:
    """Walrus allows only one sync-wait command per instruction (the
    kernel-tail drain ends up with one per DMA-completion sem lane).
    Hoist all but the last wait onto fresh single-wait NOPs inserted
    just before the instruction on the same engine — semantically
    identical (the engine blocks on each wait in turn)."""
    from bass_rust import SyncInfo

    k = 0
    for fn in nc.m.functions:
        for blk in fn.blocks:
            insts = blk.instructions
            for idx in range(len(insts) - 1, -1, -1):
                inst = insts[idx]
                si = inst.sync_info
                if si is None or len(si.on_wait) <= 1:
                    continue
                waits = list(si.on_wait)
                for w in waits[:-1]:
                    nop = mybir.InstNoOp(name=f"WSPLIT-{k}")
                    k += 1
                    nop.engine = inst.engine
                    nop.sync_info = SyncInfo(on_wait=[w], on_update=[])
                    insts.insert(idx, nop)
                si.on_wait = [waits[-1]]
                inst.sync_info = si


def get_nc() -> bass.Bass:
    if "nc" not in _NC_CACHE:
        nc = build_nc()
        _split_multi_waits(nc)
        _NC_CACHE["nc"] = nc
    return _NC_CACHE["nc"]


def make_in_maps(x: np.ndarray) -> list[dict]:
    x = np.asarray(x, dtype=np.float32)
    maps = []
    for core in range(N_CORES):
        b, half = divmod(core, 2)
        xs = x[b, half * CPC : (half + 1) * CPC]
        xp = np.pad(xs, ((0, 0), (1, PH2 - IH - 1), (1, 1))).astype(NP_DT)
        maps.append({"xp": np.ascontiguousarray(xp)})
    return maps


def gather_out(results: list[dict]) -> np.ndarray:
    out = np.empty((B, C * 9, OSZ), dtype=np.float32)
    for core in range(N_CORES):
        b, half = divmod(core, 2)
        out[b, half * NROW : (half + 1) * NROW] = results[core]["out"]
    return out


def kernel(**inputs) -> np.ndarray:
    x = inputs["x"]
    nc = get_nc()
    res = run_bass_kernel_spmd(nc, make_in_maps(x), list(range(N_CORES)))
    return gather_out(res.results)


# revision 21
# speedup vs baseline: 1.0408x; 1.0408x over previous
"""Unfold/im2col kernel for Trainium2 (Bass/Tile), 8-core data parallel.

Problem: x [4, 64, 224, 224] f32 -> out [4, 576, 49729] f32 where
out[b, (c*3+kh)*3+kw, oh*223+ow] = pad(x,1)[b, c, oh+kh, ow+kw]
(3x3 kernel, pad 1, stride 1, dilation 1, oh=ow=223 per the reference
index arithmetic).

Sharding: 8 cores = (batch 4) x (channel half 2). Each core handles
32 channels -> [288, 49729] independently; outputs concatenate on the
channel axis (channel-major row layout makes halves contiguous).

The op is pure data movement (9 shifted copies of the padded input),
so the kernel is DMA-roofline work. The 330 us baseline was SDMA-
descriptor-bound: each store descriptor was one 892 B output row
(~9 B/ns/engine). This version (measured 131-144 us, median ~132 us;
the device shows +-6 us run-to-run variance) uses three levers:

 * bf16 stores: the 2e-2 rel-err budget dwarfs bf16's 2^-9 rounding
   (measured rel err 3e-3); the host pre-casts the input and upcasts
   the gather. Write traffic halves to 28.6 MB/core.
 * On-chip repack so store descriptors are ~25 KB: the Vector engine
   packs 3 kw-crops with row stride 223 (= output row length); a
   (kh,kw,rb) plane chunk is then contiguous BOTH in a partition's
   free dim and in DRAM. 25 KB is the measured write sweet spot
   (~20 B/ns/engine; 12.7 KB and 50 KB both regress to ~13-16).
 * Load/crop0 pipelining: loads are split into two row-halves so
   crop0's first half runs on DVE while the second half still loads.

Layout per core: partition p = rb*32 + c holds row-block rb of
channel c. Row-blocks OVERLAP: block rb = padded rows [57rb, 57rb+58]
(59 rows; host pads H to 230 = 1 top + 5 bottom zero rows), so every
(kh, rb) store chunk is a uniform 57 output rows (52 for rb=3)
starting at local crop row kh. 8 load DMAs (row-half x rb, 32 x
13.3 KB descriptors each), 4 DVE crop copies, 36 store DMAs
(kw x kh x rb, 32 x ~25 KB descriptors each).

Hard-won SWDGE lore (measured on device, don't regress these):
 * The model queue (gpsimd) executes DMAs serially; descriptors of a
   DMA spread over the 16 SDMA engines BY OUTER AP DIM INDEX. Outer
   dim count must be >= 16: an rb-outer load (4 values) used only 4
   engines and took 60 us.
 * 2 descriptors/engine/DMA (32-desc DMAs) is the sweet spot. 16-desc
   DMAs are latency-bound (~3.4 us/packet); 96-desc DMAs with
   overlapping-read kh middle dims regress to ~13.5 B/ns.
 * HBM reads are flat ~13.6 B/ns/engine regardless of descriptor
   size; writes peak at ~20 B/ns at 25 KB.
 * A tile written by TWO engines makes readers fall back to
   queue-drain syncs (first store waited for crop2, +6 us). Keep each
   tile single-engine; multiple ops from ONE engine stay fine-grained.
 * Emit stores for kw right after kw's crop so waits cover only
   already-needed ops.

Phases (core 0, traced): preamble ~8 us | load 16 us | crop0 tail
~5 us | stores 96 us @ ~300 GB/s | tail ~4 us. HBM traffic/core:
3.4 MB read + 28.6 MB write.
"""

from contextlib import ExitStack

import ml_dtypes
import numpy as np

import concourse.bass as bass
import concourse.tile as tile
from concourse import mybir
from concourse.ap import AP
from concourse.bass_utils import run_bass_kernel_spmd

B, C, IH, IW = 4, 64, 224, 224
N_CORES = 8
CPC = C // 2          # channels per core: 32
PW = IW + 2           # padded width: 226
PH2 = IH + 6          # padded height incl. 5 bottom zero rows: 230
OH = IH - 1           # output spatial: 223
OSZ = OH * OH         # 49729
NROW = CPC * 9        # 288 output rows per core
RB = 4                # row-blocks per channel
RBH = 57              # block start stride (rows)
BLK = RBH + 2         # rows per block incl. 2-row overlap: 59
FRAW = BLK * PW       # 13334 raw elems per partition
FCROP = BLK * OH      # 13157 crop elems per partition
NPART = RB * CPC      # 128 partitions used
NP_DT = ml_dtypes.bfloat16
BIR_DT = mybir.dt.bfloat16

_NC_CACHE = {}


def build_nc() -> bass.Bass:
    nc = bass.Bass()
    x = nc.declare_dram_parameter("xp", [CPC, PH2, PW], BIR_DT, isOutput=False)
    out = nc.declare_dram_parameter("out", [NROW, OSZ], BIR_DT, isOutput=True)
    xb = x[:, :, :]
    ob = out[:, :]

    with tile.TileContext(nc) as tc:
        with ExitStack() as ctx:
            pool = ctx.enter_context(tc.tile_pool(name="img", bufs=1))
            raw = pool.tile([NPART, FRAW], BIR_DT, name="raw", tag="raw")[:, :]
            crops = [
                pool.tile([NPART, FCROP], BIR_DT, name=f"c{kw}", tag=f"c{kw}")[:, :]
                for kw in range(3)
            ]

            # Loads: one DMA per (row-half, rb) with CHANNELS as the
            # outer AP dim — SWDGE spreads descriptors over SDMA
            # engines by outer-dim index, so the outer count must be
            # >= 16 (an rb-outer single DMA measured 4-engines-only,
            # 60 us). Partition p = rb*32+c gets rows [57rb, 57rb+58]
            # of channel c (overlapping DRAM reads are fine). The
            # row-half split lets crop0's first half run on DVE while
            # the second half of the image still loads.
            HA = 30  # rows in first half
            for r0, nr in ((0, HA), (HA, BLK - HA)):
                for rb in range(RB):
                    nc.gpsimd.dma_start(
                        out=AP(
                            raw.tensor,
                            raw.offset + (rb * CPC) * FRAW + r0 * PW,
                            [[FRAW, CPC], [1, nr * PW]],
                        ),
                        in_=AP(
                            xb.tensor,
                            xb.offset + (rb * RBH + r0) * PW,
                            [[PH2 * PW, CPC], [1, nr * PW]],
                        ),
                    )

            # Shift-pack the 3 kw-crops (row stride 226 -> 223).
            # crop0 gates the first stores: split by rows over
            # DVE + Scalar. Stores for each kw are EMITTED right after
            # that kw's crop so any conservative queue-drain sync only
            # covers already-needed ops. kw=1 runs on Scalar, kw=2 on
            # DVE, both hidden under the kw=0 store burst.
            def cap(t, free, kw_off, pitch, r0, nr):
                return AP(
                    t.tensor,
                    t.offset + kw_off + r0 * (OH if pitch == OH else PW),
                    [[free, NPART], [pitch, nr], [1, OH]],
                )

            def emit_stores(kw):
                ck = crops[kw]
                for kh in range(3):
                    for rb in range(RB):
                        nrows = RBH if rb < RB - 1 else OH - RBH * (RB - 1)
                        src = AP(
                            ck.tensor,
                            ck.offset + (rb * CPC) * FCROP + kh * OH,
                            [[FCROP, CPC], [1, nrows * OH]],
                        )
                        dst = AP(
                            ob.tensor,
                            ob.offset + (kh * 3 + kw) * OSZ + (rb * RBH) * OH,
                            [[9 * OSZ, CPC], [1, nrows * OH]],
                        )
                        nc.gpsimd.dma_start(out=dst, in_=src)

            # All crops single-writer on DVE: a tile written by two
            # ENGINES makes readers fall back to queue-drain syncs
            # (measured: first store waited for crop2's drain, +6 us).
            # Two DVE ops on one tile keep fine-grained op-count waits.
            # crop0's first half starts right after the first-half
            # loads, overlapping the second-half loads.
            nc.vector.tensor_copy(
                out=cap(crops[0], FCROP, 0, OH, 0, HA),
                in_=cap(raw, FRAW, 0, PW, 0, HA),
            )
            nc.vector.tensor_copy(
                out=cap(crops[0], FCROP, 0, OH, HA, BLK - HA),
                in_=cap(raw, FRAW, 0, PW, HA, BLK - HA),
            )
            emit_stores(0)
            for kw in (1, 2):
                nc.vector.tensor_copy(
                    out=cap(crops[kw], FCROP, 0, OH, 0, BLK),
                    in_=cap(raw, FRAW, kw, PW, 0, BLK),
                )
                emit_stores(kw)
    return nc


def _split_multi_waits(nc: bass.Bass) -> None:
    """Walrus allows only one sync-wait command per instruction (the
    kernel-tail drain ends up with one per DMA-completion sem lane).
    Hoist all but the last wait onto fresh single-wait NOPs inserted
    just before the instruction on the same engine — semantically
    identical (the engine blocks on each wait in turn)."""
    from bass_rust import SyncInfo

    k = 0
    for fn in nc.m.functions:
        for blk in fn.blocks:
            insts = blk.instructions
            for idx in range(len(insts) - 1, -1, -1):
                inst = insts[idx]
                si = inst.sync_info
                if si is None or len(si.on_wait) <= 1:
                    continue
                waits = list(si.on_wait)
                for w in waits[:-1]:
                    nop = mybir.InstNoOp(name=f"WSPLIT-{k}")
                    k += 1
                    nop.engine = inst.engine
                    nop.sync_info = SyncInfo(on_wait=[w], on_update=[])
                    insts.insert(idx, nop)
                si.on_wait = [waits[-1]]
                inst.sync_info = si


def get_nc() -> bass.Bass:
    if "nc" not in _NC_CACHE:
        nc = build_nc()
        _split_multi_waits(nc)
        _NC_CACHE["nc"] = nc
    return _NC_CACHE["nc"]


def make_in_maps(x: np.ndarray) -> list[dict]:
    x = np.asarray(x, dtype=np.float32)
    maps = []
    for core in range(N_CORES):
        b, half = divmod(core, 2)
        xs = x[b, half * CPC : (half + 1) * CPC]
        xp = np.pad(xs, ((0, 0), (1, PH2 - IH - 1), (1, 1))).astype(NP_DT)
        maps.append({"xp": np.ascontiguousarray(xp)})
    return maps


def gather_out(results: list[dict]) -> np.ndarray:
    out = np.empty((B, C * 9, OSZ), dtype=np.float32)
    for core in range(N_CORES):
        b, half = divmod(core, 2)
        out[b, half * NROW : (half + 1) * NROW] = results[core]["out"]
    return out


def kernel(**inputs) -> np.ndarray:
    x = inputs["x"]
    nc = get_nc()
    res = run_bass_kernel_spmd(nc, make_in_maps(x), list(range(N_CORES)))
    return gather_out(res.results)


# revision 22
# speedup vs baseline: 1.0430x; 1.0021x over previous
"""Unfold/im2col kernel for Trainium2 (Bass/Tile), 8-core data parallel.

Problem: x [4, 64, 224, 224] f32 -> out [4, 576, 49729] f32 where
out[b, (c*3+kh)*3+kw, oh*223+ow] = pad(x,1)[b, c, oh+kh, ow+kw]
(3x3 kernel, pad 1, stride 1, dilation 1, oh=ow=223 per the reference
index arithmetic).

Sharding: 8 cores = (batch 4) x (channel half 2). Each core handles
32 channels -> [288, 49729] independently; outputs concatenate on the
channel axis (channel-major row layout makes halves contiguous).

The op is pure data movement (9 shifted copies of the padded input),
so the kernel is DMA-roofline work. The 330 us baseline was SDMA-
descriptor-bound: each store descriptor was one 892 B output row
(~9 B/ns/engine). This version (measured 131-144 us, median ~132 us;
the device shows +-6 us run-to-run variance) uses three levers:

 * bf16 stores: the 2e-2 rel-err budget dwarfs bf16's 2^-9 rounding
   (measured rel err 3e-3); the host pre-casts the input and upcasts
   the gather. Write traffic halves to 28.6 MB/core.
 * On-chip repack so store descriptors are ~25 KB: the Vector engine
   packs 3 kw-crops with row stride 223 (= output row length); a
   (kh,kw,rb) plane chunk is then contiguous BOTH in a partition's
   free dim and in DRAM. 25 KB is the measured write sweet spot
   (~20 B/ns/engine; 12.7 KB and 50 KB both regress to ~13-16).
 * Load/crop0 pipelining: loads are split into two row-halves so
   crop0's first half runs on DVE while the second half still loads.

Layout per core: partition p = rb*32 + c holds row-block rb of
channel c. Row-blocks OVERLAP: block rb = padded rows [57rb, 57rb+58]
(59 rows; host pads H to 230 = 1 top + 5 bottom zero rows), so every
(kh, rb) store chunk is a uniform 57 output rows (52 for rb=3)
starting at local crop row kh. 8 load DMAs (row-half x rb, 32 x
13.3 KB descriptors each), 4 DVE crop copies, 36 store DMAs
(kw x kh x rb, 32 x ~25 KB descriptors each).

Hard-won SWDGE lore (measured on device, don't regress these):
 * The model queue (gpsimd) executes DMAs serially; descriptors of a
   DMA spread over the 16 SDMA engines BY OUTER AP DIM INDEX. Outer
   dim count must be >= 16: an rb-outer load (4 values) used only 4
   engines and took 60 us.
 * 2 descriptors/engine/DMA (32-desc DMAs) is the sweet spot. 16-desc
   DMAs are latency-bound (~3.4 us/packet); 96-desc DMAs with
   overlapping-read kh middle dims regress to ~13.5 B/ns.
 * HBM reads are flat ~13.6 B/ns/engine regardless of descriptor
   size; writes peak at ~20 B/ns at 25 KB.
 * A tile written by TWO engines makes readers fall back to
   queue-drain syncs (first store waited for crop2, +6 us). Keep each
   tile single-engine; multiple ops from ONE engine stay fine-grained.
 * Emit stores for kw right after kw's crop so waits cover only
   already-needed ops.

Phases (core 0, traced): preamble ~8 us | load 16 us | crop0 tail
~5 us | stores 96 us @ ~300 GB/s | tail ~4 us. HBM traffic/core:
3.4 MB read + 28.6 MB write.
"""

from contextlib import ExitStack

import ml_dtypes
import numpy as np

import concourse.bass as bass
import concourse.tile as tile
from concourse import mybir
from concourse.ap import AP
from concourse.bass_utils import run_bass_kernel_spmd

B, C, IH, IW = 4, 64, 224, 224
N_CORES = 8
CPC = C // 2          # channels per core: 32
PW = IW + 2           # padded width: 226
PH2 = IH + 7          # padded height incl. 6 bottom zero rows: 231
OH = IH - 1           # output spatial: 223
OSZ = OH * OH         # 49729
NROW = CPC * 9        # 288 output rows per core
RB = 4                # row-blocks per channel
RBH = 57              # block start stride (rows)
BLK = RBH + 3         # rows per block incl. 3-row overlap: 60 (even: DVE 2x)
FRAW = BLK * PW       # 13560 raw elems per partition
FCROP = BLK * OH      # 13380 crop elems per partition
NPART = RB * CPC      # 128 partitions used
NP_DT = ml_dtypes.bfloat16
BIR_DT = mybir.dt.bfloat16

_NC_CACHE = {}


def build_nc() -> bass.Bass:
    nc = bass.Bass()
    x = nc.declare_dram_parameter("xp", [CPC, PH2, PW], BIR_DT, isOutput=False)
    out = nc.declare_dram_parameter("out", [NROW, OSZ], BIR_DT, isOutput=True)
    xb = x[:, :, :]
    ob = out[:, :]

    with tile.TileContext(nc) as tc:
        with ExitStack() as ctx:
            pool = ctx.enter_context(tc.tile_pool(name="img", bufs=1))
            raw = pool.tile([NPART, FRAW], BIR_DT, name="raw", tag="raw")[:, :]
            crops = [
                pool.tile([NPART, FCROP], BIR_DT, name=f"c{kw}", tag=f"c{kw}")[:, :]
                for kw in range(3)
            ]

            # Loads: one DMA per (row-half, rb) with CHANNELS as the
            # outer AP dim — SWDGE spreads descriptors over SDMA
            # engines by outer-dim index, so the outer count must be
            # >= 16 (an rb-outer single DMA measured 4-engines-only,
            # 60 us). Partition p = rb*32+c gets rows [57rb, 57rb+58]
            # of channel c (overlapping DRAM reads are fine). The
            # row-half split lets crop0's first half run on DVE while
            # the second half of the image still loads.
            HA = 32  # rows in first half (even count + 32B-aligned split: 32*452B = 32B multiple)
            for r0, nr in ((0, HA), (HA, BLK - HA)):
                for rb in range(RB):
                    nc.gpsimd.dma_start(
                        out=AP(
                            raw.tensor,
                            raw.offset + (rb * CPC) * FRAW + r0 * PW,
                            [[FRAW, CPC], [1, nr * PW]],
                        ),
                        in_=AP(
                            xb.tensor,
                            xb.offset + (rb * RBH + r0) * PW,
                            [[PH2 * PW, CPC], [1, nr * PW]],
                        ),
                    )

            # Shift-pack the 3 kw-crops (row stride 226 -> 223).
            # crop0 gates the first stores: split by rows over
            # DVE + Scalar. Stores for each kw are EMITTED right after
            # that kw's crop so any conservative queue-drain sync only
            # covers already-needed ops. kw=1 runs on Scalar, kw=2 on
            # DVE, both hidden under the kw=0 store burst.
            def cap(t, free, kw_off, pitch, r0, nr):
                return AP(
                    t.tensor,
                    t.offset + kw_off + r0 * (OH if pitch == OH else PW),
                    [[free, NPART], [pitch, nr], [1, OH]],
                )

            def emit_stores(kw):
                ck = crops[kw]
                for kh in range(3):
                    for rb in range(RB):
                        nrows = RBH if rb < RB - 1 else OH - RBH * (RB - 1)
                        src = AP(
                            ck.tensor,
                            ck.offset + (rb * CPC) * FCROP + kh * OH,
                            [[FCROP, CPC], [1, nrows * OH]],
                        )
                        dst = AP(
                            ob.tensor,
                            ob.offset + (kh * 3 + kw) * OSZ + (rb * RBH) * OH,
                            [[9 * OSZ, CPC], [1, nrows * OH]],
                        )
                        nc.gpsimd.dma_start(out=dst, in_=src)

            # All crops single-writer on DVE: a tile written by two
            # ENGINES makes readers fall back to queue-drain syncs
            # (measured: first store waited for crop2's drain, +6 us).
            # Two DVE ops on one tile keep fine-grained op-count waits.
            # crop0's first half starts right after the first-half
            # loads, overlapping the second-half loads.
            nc.vector.tensor_copy(
                out=cap(crops[0], FCROP, 0, OH, 0, HA),
                in_=cap(raw, FRAW, 0, PW, 0, HA),
            )
            nc.vector.tensor_copy(
                out=cap(crops[0], FCROP, 0, OH, HA, BLK - HA),
                in_=cap(raw, FRAW, 0, PW, HA, BLK - HA),
            )
            emit_stores(0)
            for kw in (1, 2):
                nc.vector.tensor_copy(
                    out=cap(crops[kw], FCROP, 0, OH, 0, BLK),
                    in_=cap(raw, FRAW, kw, PW, 0, BLK),
                )
                emit_stores(kw)
    return nc


def _split_multi_waits(nc: bass.Bass) -> None:
    """Walrus allows only one sync-wait command per instruction (the
    kernel-tail drain ends up with one per DMA-completion sem lane).
    Hoist all but the last wait onto fresh single-wait NOPs inserted
    just before the instruction on the same engine — semantically
    identical (the engine blocks on each wait in turn)."""
    from bass_rust import SyncInfo

    k = 0
    for fn in nc.m.functions:
        for blk in fn.blocks:
            insts = blk.instructions
            for idx in range(len(insts) - 1, -1, -1):
                inst = insts[idx]
                si = inst.sync_info
                if si is None or len(si.on_wait) <= 1:
                    continue
                waits = list(si.on_wait)
                for w in waits[:-1]:
                    nop = mybir.InstNoOp(name=f"WSPLIT-{k}")
                    k += 1
                    nop.engine = inst.engine
                    nop.sync_info = SyncInfo(on_wait=[w], on_update=[])
                    insts.insert(idx, nop)
                si.on_wait = [waits[-1]]
                inst.sync_info = si


def get_nc() -> bass.Bass:
    if "nc" not in _NC_CACHE:
        nc = build_nc()
        _split_multi_waits(nc)
        _NC_CACHE["nc"] = nc
    return _NC_CACHE["nc"]


def make_in_maps(x: np.ndarray) -> list[dict]:
    x = np.asarray(x, dtype=np.float32)
    maps = []
    for core in range(N_CORES):
        b, half = divmod(core, 2)
        xs = x[b, half * CPC : (half + 1) * CPC]
        xp = np.pad(xs, ((0, 0), (1, PH2 - IH - 1), (1, 1))).astype(NP_DT)
        maps.append({"xp": np.ascontiguousarray(xp)})
    return maps


def gather_out(results: list[dict]) -> np.ndarray:
    out = np.empty((B, C * 9, OSZ), dtype=np.float32)
    for core in range(N_CORES):
        b, half = divmod(core, 2)
        out[b, half * NROW : (half + 1) * NROW] = results[core]["out"]
    return out


def kernel(**inputs) -> np.ndarray:
    x = inputs["x"]
    nc = get_nc()
    res = run_bass_kernel_spmd(nc, make_in_maps(x), list(range(N_CORES)))
    return gather_out(res.results)


# revision 23
# speedup vs baseline: 1.0684x; 1.0243x over previous
"""Unfold/im2col kernel for Trainium2 (Bass/Tile), 8-core data parallel.

Problem: x [4, 64, 224, 224] f32 -> out [4, 576, 49729] f32 where
out[b, (c*3+kh)*3+kw, oh*223+ow] = pad(x,1)[b, c, oh+kh, ow+kw]
(3x3 kernel, pad 1, stride 1, dilation 1, oh=ow=223 per the reference
index arithmetic).

Sharding: 8 cores = (batch 4) x (channel half 2). Each core handles
32 channels -> [288, 49729] independently; outputs concatenate on the
channel axis (channel-major row layout makes halves contiguous).

The op is pure data movement (9 shifted copies of the padded input),
so the kernel is DMA-roofline work. The 330 us baseline was SDMA-
descriptor-bound: each store descriptor was one 892 B output row
(~9 B/ns/engine). This version (measured 131-144 us, median ~132 us;
the device shows +-6 us run-to-run variance) uses three levers:

 * bf16 stores: the 2e-2 rel-err budget dwarfs bf16's 2^-9 rounding
   (measured rel err 3e-3); the host pre-casts the input and upcasts
   the gather. Write traffic halves to 28.6 MB/core.
 * On-chip repack so store descriptors are ~25 KB: the Vector engine
   packs 3 kw-crops with row stride 223 (= output row length); a
   (kh,kw,rb) plane chunk is then contiguous BOTH in a partition's
   free dim and in DRAM. 25 KB is the measured write sweet spot
   (~20 B/ns/engine; 12.7 KB and 50 KB both regress to ~13-16).
 * Load/crop0 pipelining: loads are split into two row-halves so
   crop0's first half runs on DVE while the second half still loads.

Layout per core: partition p = rb*32 + c holds row-block rb of
channel c. Row-blocks OVERLAP: block rb = padded rows [57rb, 57rb+58]
(59 rows; host pads H to 230 = 1 top + 5 bottom zero rows), so every
(kh, rb) store chunk is a uniform 57 output rows (52 for rb=3)
starting at local crop row kh. 8 load DMAs (row-half x rb, 32 x
13.3 KB descriptors each), 4 DVE crop copies, 36 store DMAs
(kw x kh x rb, 32 x ~25 KB descriptors each).

Hard-won SWDGE lore (measured on device, don't regress these):
 * The model queue (gpsimd) executes DMAs serially; descriptors of a
   DMA spread over the 16 SDMA engines BY OUTER AP DIM INDEX. Outer
   dim count must be >= 16: an rb-outer load (4 values) used only 4
   engines and took 60 us.
 * 2 descriptors/engine/DMA (32-desc DMAs) is the sweet spot. 16-desc
   DMAs are latency-bound (~3.4 us/packet); 96-desc DMAs with
   overlapping-read kh middle dims regress to ~13.5 B/ns.
 * HBM reads are flat ~13.6 B/ns/engine regardless of descriptor
   size; writes peak at ~20 B/ns at 25 KB.
 * A tile written by TWO engines makes readers fall back to
   queue-drain syncs (first store waited for crop2, +6 us). Keep each
   tile single-engine; multiple ops from ONE engine stay fine-grained.
 * Emit stores for kw right after kw's crop so waits cover only
   already-needed ops.

Phases (core 0, traced): preamble ~8 us | load 16 us | crop0 tail
~5 us | stores 96 us @ ~300 GB/s | tail ~4 us. HBM traffic/core:
3.4 MB read + 28.6 MB write.
"""

from contextlib import ExitStack

import ml_dtypes
import numpy as np

import concourse.bass as bass
import concourse.tile as tile
from concourse import mybir
from concourse.ap import AP
from concourse.bass_utils import run_bass_kernel_spmd

B, C, IH, IW = 4, 64, 224, 224
N_CORES = 8
CPC = C // 2          # channels per core: 32
PW = IW + 2           # padded width: 226
PH2 = IH + 7          # padded height incl. 6 bottom zero rows: 231
OH = IH - 1           # output spatial: 223
OSZ = OH * OH         # 49729
NROW = CPC * 9        # 288 output rows per core
RB = 4                # row-blocks per channel
RBH = 57              # block start stride (rows)
BLK = RBH + 3         # rows per block incl. 3-row overlap: 60 (even: DVE 2x)
FRAW = BLK * PW       # 13560 raw elems per partition
FCROP = BLK * OH      # 13380 crop elems per partition
NPART = RB * CPC      # 128 partitions used
NP_DT = ml_dtypes.bfloat16
BIR_DT = mybir.dt.bfloat16

_NC_CACHE = {}


def build_nc() -> bass.Bass:
    nc = bass.Bass()
    x = nc.declare_dram_parameter("xp", [CPC, PH2, PW], BIR_DT, isOutput=False)
    out = nc.declare_dram_parameter("out", [NROW, OSZ], BIR_DT, isOutput=True)
    xb = x[:, :, :]
    ob = out[:, :]

    with tile.TileContext(nc) as tc:
        with ExitStack() as ctx:
            pool = ctx.enter_context(tc.tile_pool(name="img", bufs=1))
            raw = pool.tile([NPART, FRAW], BIR_DT, name="raw", tag="raw")[:, :]
            crops = [
                pool.tile([NPART, FCROP], BIR_DT, name=f"c{kw}", tag=f"c{kw}")[:, :]
                for kw in range(3)
            ]

            # Loads: one DMA per (row-half, rb) with CHANNELS as the
            # outer AP dim — SWDGE spreads descriptors over SDMA
            # engines by outer-dim index, so the outer count must be
            # >= 16 (an rb-outer single DMA measured 4-engines-only,
            # 60 us). Partition p = rb*32+c gets rows [57rb, 57rb+58]
            # of channel c (overlapping DRAM reads are fine). The
            # row-half split lets crop0's first half run on DVE while
            # the second half of the image still loads.
            HA = 32  # rows in first half (even count + 32B-aligned split: 32*452B = 32B multiple)
            for r0, nr in ((0, HA), (HA, BLK - HA)):
                for rb in range(RB):
                    nc.gpsimd.dma_start(
                        out=AP(
                            raw.tensor,
                            raw.offset + (rb * CPC) * FRAW + r0 * PW,
                            [[FRAW, CPC], [1, nr * PW]],
                        ),
                        in_=AP(
                            xb.tensor,
                            xb.offset + (rb * RBH + r0) * PW,
                            [[PH2 * PW, CPC], [1, nr * PW]],
                        ),
                    )

            # Shift-pack the 3 kw-crops (row stride 226 -> 223).
            # crop0 gates the first stores: split by rows over
            # DVE + Scalar. Stores for each kw are EMITTED right after
            # that kw's crop so any conservative queue-drain sync only
            # covers already-needed ops. kw=1 runs on Scalar, kw=2 on
            # DVE, both hidden under the kw=0 store burst.
            def cap(t, free, kw_off, pitch, r0, nr):
                return AP(
                    t.tensor,
                    t.offset + kw_off + r0 * (OH if pitch == OH else PW),
                    [[free, NPART], [pitch, nr], [1, OH]],
                )

            # Two kw=0 chunks ride the otherwise-idle HWDGE rings
            # (sync/scalar engines, ~15-28 GB/s each, ~40 us per
            # 0.81 MB chunk) concurrently with the whole SWDGE store
            # phase, shaving their bytes off the serial model queue.
            RING = {(0, 2, 1): "sync", (0, 2, 2): "scalar"}

            def emit_stores(kw):
                ck = crops[kw]
                for kh in range(3):
                    for rb in range(RB):
                        nrows = RBH if rb < RB - 1 else OH - RBH * (RB - 1)
                        src = AP(
                            ck.tensor,
                            ck.offset + (rb * CPC) * FCROP + kh * OH,
                            [[FCROP, CPC], [1, nrows * OH]],
                        )
                        dst = AP(
                            ob.tensor,
                            ob.offset + (kh * 3 + kw) * OSZ + (rb * RBH) * OH,
                            [[9 * OSZ, CPC], [1, nrows * OH]],
                        )
                        eng = getattr(nc, RING.get((kw, kh, rb), "gpsimd"))
                        eng.dma_start(out=dst, in_=src)

            # All crops single-writer on DVE: a tile written by two
            # ENGINES makes readers fall back to queue-drain syncs
            # (measured: first store waited for crop2's drain, +6 us).
            # Two DVE ops on one tile keep fine-grained op-count waits.
            # crop0's first half starts right after the first-half
            # loads, overlapping the second-half loads.
            nc.vector.tensor_copy(
                out=cap(crops[0], FCROP, 0, OH, 0, HA),
                in_=cap(raw, FRAW, 0, PW, 0, HA),
            )
            nc.vector.tensor_copy(
                out=cap(crops[0], FCROP, 0, OH, HA, BLK - HA),
                in_=cap(raw, FRAW, 0, PW, HA, BLK - HA),
            )
            emit_stores(0)
            for kw in (1, 2):
                nc.vector.tensor_copy(
                    out=cap(crops[kw], FCROP, 0, OH, 0, BLK),
                    in_=cap(raw, FRAW, kw, PW, 0, BLK),
                )
                emit_stores(kw)
    return nc


def _split_multi_waits(nc: bass.Bass) -> None:
    """Walrus allows only one sync-wait command per instruction (the
    kernel-tail drain ends up with one per DMA-completion sem lane).
    Hoist all but the last wait onto fresh single-wait NOPs inserted
    just before the instruction on the same engine — semantically
    identical (the engine blocks on each wait in turn)."""
    from bass_rust import SyncInfo

    k = 0
    for fn in nc.m.functions:
        for blk in fn.blocks:
            insts = blk.instructions
            for idx in range(len(insts) - 1, -1, -1):
                inst = insts[idx]
                si = inst.sync_info
                if si is None or len(si.on_wait) <= 1:
                    continue
                waits = list(si.on_wait)
                for w in waits[:-1]:
                    nop = mybir.InstNoOp(name=f"WSPLIT-{k}")
                    k += 1
                    nop.engine = inst.engine
                    nop.sync_info = SyncInfo(on_wait=[w], on_update=[])
                    insts.insert(idx, nop)
                si.on_wait = [waits[-1]]
                inst.sync_info = si


def get_nc() -> bass.Bass:
    if "nc" not in _NC_CACHE:
        nc = build_nc()
        _split_multi_waits(nc)
        _NC_CACHE["nc"] = nc
    return _NC_CACHE["nc"]


def make_in_maps(x: np.ndarray) -> list[dict]:
    x = np.asarray(x, dtype=np.float32)
    maps = []
    for core in range(N_CORES):
        b, half = divmod(core, 2)
        xs = x[b, half * CPC : (half + 1) * CPC]
        xp = np.pad(xs, ((0, 0), (1, PH2 - IH - 1), (1, 1))).astype(NP_DT)
        maps.append({"xp": np.ascontiguousarray(xp)})
    return maps


def gather_out(results: list[dict]) -> np.ndarray:
    out = np.empty((B, C * 9, OSZ), dtype=np.float32)
    for core in range(N_CORES):
        b, half = divmod(core, 2)
        out[b, half * NROW : (half + 1) * NROW] = results[core]["out"]
    return out


def kernel(**inputs) -> np.ndarray:
    x = inputs["x"]
    nc = get_nc()
    res = run_bass_kernel_spmd(nc, make_in_maps(x), list(range(N_CORES)))
    return gather_out(res.results)


# revision 24
# speedup vs baseline: 1.0799x; 1.0108x over previous
"""Unfold/im2col kernel for Trainium2 (Bass/Tile), 8-core data parallel.

Problem: x [4, 64, 224, 224] f32 -> out [4, 576, 49729] f32 where
out[b, (c*3+kh)*3+kw, oh*223+ow] = pad(x,1)[b, c, oh+kh, ow+kw]
(3x3 kernel, pad 1, stride 1, dilation 1, oh=ow=223 per the reference
index arithmetic).

Sharding: 8 cores = (batch 4) x (channel half 2). Each core handles
32 channels -> [288, 49729] independently; outputs concatenate on the
channel axis (channel-major row layout makes halves contiguous).

The op is pure data movement (9 shifted copies of the padded input),
so the kernel is DMA-roofline work. The 330 us baseline was SDMA-
descriptor-bound: each store descriptor was one 892 B output row
(~9 B/ns/engine). This version (measured 131-144 us, median ~132 us;
the device shows +-6 us run-to-run variance) uses three levers:

 * bf16 stores: the 2e-2 rel-err budget dwarfs bf16's 2^-9 rounding
   (measured rel err 3e-3); the host pre-casts the input and upcasts
   the gather. Write traffic halves to 28.6 MB/core.
 * On-chip repack so store descriptors are ~25 KB: the Vector engine
   packs 3 kw-crops with row stride 223 (= output row length); a
   (kh,kw,rb) plane chunk is then contiguous BOTH in a partition's
   free dim and in DRAM. 25 KB is the measured write sweet spot
   (~20 B/ns/engine; 12.7 KB and 50 KB both regress to ~13-16).
 * Load/crop0 pipelining: loads are split into two row-halves so
   crop0's first half runs on DVE while the second half still loads.

Layout per core: partition p = rb*32 + c holds row-block rb of
channel c. Row-blocks OVERLAP: block rb = padded rows [57rb, 57rb+58]
(59 rows; host pads H to 230 = 1 top + 5 bottom zero rows), so every
(kh, rb) store chunk is a uniform 57 output rows (52 for rb=3)
starting at local crop row kh. 8 load DMAs (row-half x rb, 32 x
13.3 KB descriptors each), 4 DVE crop copies, 36 store DMAs
(kw x kh x rb, 32 x ~25 KB descriptors each).

Hard-won SWDGE lore (measured on device, don't regress these):
 * The model queue (gpsimd) executes DMAs serially; descriptors of a
   DMA spread over the 16 SDMA engines BY OUTER AP DIM INDEX. Outer
   dim count must be >= 16: an rb-outer load (4 values) used only 4
   engines and took 60 us.
 * 2 descriptors/engine/DMA (32-desc DMAs) is the sweet spot. 16-desc
   DMAs are latency-bound (~3.4 us/packet); 96-desc DMAs with
   overlapping-read kh middle dims regress to ~13.5 B/ns.
 * HBM reads are flat ~13.6 B/ns/engine regardless of descriptor
   size; writes peak at ~20 B/ns at 25 KB.
 * A tile written by TWO engines makes readers fall back to
   queue-drain syncs (first store waited for crop2, +6 us). Keep each
   tile single-engine; multiple ops from ONE engine stay fine-grained.
 * Emit stores for kw right after kw's crop so waits cover only
   already-needed ops.

Phases (core 0, traced): preamble ~8 us | load 16 us | crop0 tail
~5 us | stores 96 us @ ~300 GB/s | tail ~4 us. HBM traffic/core:
3.4 MB read + 28.6 MB write.
"""

from contextlib import ExitStack

import ml_dtypes
import numpy as np

import concourse.bass as bass
import concourse.tile as tile
from concourse import mybir
from concourse.ap import AP
from concourse.bass_utils import run_bass_kernel_spmd

B, C, IH, IW = 4, 64, 224, 224
N_CORES = 8
CPC = C // 2          # channels per core: 32
PW = IW + 2           # padded width: 226
PH2 = IH + 7          # padded height incl. 6 bottom zero rows: 231
OH = IH - 1           # output spatial: 223
OSZ = OH * OH         # 49729
NROW = CPC * 9        # 288 output rows per core
RB = 4                # row-blocks per channel
RBH = 57              # block start stride (rows)
BLK = RBH + 3         # rows per block incl. 3-row overlap: 60 (even: DVE 2x)
FRAW = BLK * PW       # 13560 raw elems per partition
FCROP = BLK * OH      # 13380 crop elems per partition
NPART = RB * CPC      # 128 partitions used
NP_DT = ml_dtypes.bfloat16
BIR_DT = mybir.dt.bfloat16

_NC_CACHE = {}


def build_nc() -> bass.Bass:
    nc = bass.Bass()
    x = nc.declare_dram_parameter("xp", [CPC, PH2, PW], BIR_DT, isOutput=False)
    out = nc.declare_dram_parameter("out", [NROW, OSZ], BIR_DT, isOutput=True)
    xb = x[:, :, :]
    ob = out[:, :]

    with tile.TileContext(nc) as tc:
        with ExitStack() as ctx:
            pool = ctx.enter_context(tc.tile_pool(name="img", bufs=1))
            raw = pool.tile([NPART, FRAW], BIR_DT, name="raw", tag="raw")[:, :]
            crops = [
                pool.tile([NPART, FCROP], BIR_DT, name=f"c{kw}", tag=f"c{kw}")[:, :]
                for kw in range(3)
            ]

            # Loads: one DMA per (row-half, rb) with CHANNELS as the
            # outer AP dim — SWDGE spreads descriptors over SDMA
            # engines by outer-dim index, so the outer count must be
            # >= 16 (an rb-outer single DMA measured 4-engines-only,
            # 60 us). Partition p = rb*32+c gets rows [57rb, 57rb+58]
            # of channel c (overlapping DRAM reads are fine). The
            # row-half split lets crop0's first half run on DVE while
            # the second half of the image still loads.
            HA = 32  # rows in first half (even count + 32B-aligned split: 32*452B = 32B multiple)
            LRING = {(0, 3): "sync", (HA, 3): "scalar"}
            for r0, nr in ((0, HA), (HA, BLK - HA)):
                for rb in range(RB):
                    getattr(nc, LRING.get((r0, rb), "gpsimd")).dma_start(
                        out=AP(
                            raw.tensor,
                            raw.offset + (rb * CPC) * FRAW + r0 * PW,
                            [[FRAW, CPC], [1, nr * PW]],
                        ),
                        in_=AP(
                            xb.tensor,
                            xb.offset + (rb * RBH + r0) * PW,
                            [[PH2 * PW, CPC], [1, nr * PW]],
                        ),
                    )

            # Shift-pack the 3 kw-crops (row stride 226 -> 223).
            # crop0 gates the first stores: split by rows over
            # DVE + Scalar. Stores for each kw are EMITTED right after
            # that kw's crop so any conservative queue-drain sync only
            # covers already-needed ops. kw=1 runs on Scalar, kw=2 on
            # DVE, both hidden under the kw=0 store burst.
            def cap(t, free, kw_off, pitch, r0, nr):
                return AP(
                    t.tensor,
                    t.offset + kw_off + r0 * (OH if pitch == OH else PW),
                    [[free, NPART], [pitch, nr], [1, OH]],
                )

            # Two kw=0 chunks ride the otherwise-idle HWDGE rings
            # (sync/scalar engines, ~15-28 GB/s each, ~40 us per
            # 0.81 MB chunk) concurrently with the whole SWDGE store
            # phase, shaving their bytes off the serial model queue.
            RING = {(0, 2, 1): "sync", (0, 2, 2): "scalar"}

            def emit_stores(kw):
                ck = crops[kw]
                for kh in range(3):
                    for rb in range(RB):
                        nrows = RBH if rb < RB - 1 else OH - RBH * (RB - 1)
                        src = AP(
                            ck.tensor,
                            ck.offset + (rb * CPC) * FCROP + kh * OH,
                            [[FCROP, CPC], [1, nrows * OH]],
                        )
                        dst = AP(
                            ob.tensor,
                            ob.offset + (kh * 3 + kw) * OSZ + (rb * RBH) * OH,
                            [[9 * OSZ, CPC], [1, nrows * OH]],
                        )
                        eng = getattr(nc, RING.get((kw, kh, rb), "gpsimd"))
                        eng.dma_start(out=dst, in_=src)

            # All crops single-writer on DVE: a tile written by two
            # ENGINES makes readers fall back to queue-drain syncs
            # (measured: first store waited for crop2's drain, +6 us).
            # Two DVE ops on one tile keep fine-grained op-count waits.
            # crop0's first half starts right after the first-half
            # loads, overlapping the second-half loads.
            nc.vector.tensor_copy(
                out=cap(crops[0], FCROP, 0, OH, 0, HA),
                in_=cap(raw, FRAW, 0, PW, 0, HA),
            )
            nc.vector.tensor_copy(
                out=cap(crops[0], FCROP, 0, OH, HA, BLK - HA),
                in_=cap(raw, FRAW, 0, PW, HA, BLK - HA),
            )
            emit_stores(0)
            for kw in (1, 2):
                nc.vector.tensor_copy(
                    out=cap(crops[kw], FCROP, 0, OH, 0, BLK),
                    in_=cap(raw, FRAW, kw, PW, 0, BLK),
                )
                emit_stores(kw)
    return nc


def _split_multi_waits(nc: bass.Bass) -> None:
    """Walrus allows only one sync-wait command per instruction (the
    kernel-tail drain ends up with one per DMA-completion sem lane).
    Hoist all but the last wait onto fresh single-wait NOPs inserted
    just before the instruction on the same engine — semantically
    identical (the engine blocks on each wait in turn)."""
    from bass_rust import SyncInfo

    k = 0
    for fn in nc.m.functions:
        for blk in fn.blocks:
            insts = blk.instructions
            for idx in range(len(insts) - 1, -1, -1):
                inst = insts[idx]
                si = inst.sync_info
                if si is None or len(si.on_wait) <= 1:
                    continue
                waits = list(si.on_wait)
                for w in waits[:-1]:
                    nop = mybir.InstNoOp(name=f"WSPLIT-{k}")
                    k += 1
                    nop.engine = inst.engine
                    nop.sync_info = SyncInfo(on_wait=[w], on_update=[])
                    insts.insert(idx, nop)
                si.on_wait = [waits[-1]]
                inst.sync_info = si


def get_nc() -> bass.Bass:
    if "nc" not in _NC_CACHE:
        nc = build_nc()
        _split_multi_waits(nc)
        _NC_CACHE["nc"] = nc
    return _NC_CACHE["nc"]


def make_in_maps(x: np.ndarray) -> list[dict]:
    x = np.asarray(x, dtype=np.float32)
    maps = []
    for core in range(N_CORES):
        b, half = divmod(core, 2)
        xs = x[b, half * CPC : (half + 1) * CPC]
        xp = np.pad(xs, ((0, 0), (1, PH2 - IH - 1), (1, 1))).astype(NP_DT)
        maps.append({"xp": np.ascontiguousarray(xp)})
    return maps


def gather_out(results: list[dict]) -> np.ndarray:
    out = np.empty((B, C * 9, OSZ), dtype=np.float32)
    for core in range(N_CORES):
        b, half = divmod(core, 2)
        out[b, half * NROW : (half + 1) * NROW] = results[core]["out"]
    return out


def kernel(**inputs) -> np.ndarray:
    x = inputs["x"]
    nc = get_nc()
    res = run_bass_kernel_spmd(nc, make_in_maps(x), list(range(N_CORES)))
    return gather_out(res.results)
